# revision 18
# baseline (speedup 1.0000x reference)
"""Trainium2 Bass kernel for nn_AA_encoder (gnn_message_passing).

v3: activity-sparse data parallelism.  A graph's output rows are all
zero unless aa_graph_length > 64 (clause nodes 64..127 are otherwise
invalid, so adj rows vanish and gcn_out = relu(gcn_b)=0).  Only the
~29/64 active graphs are computed: the host sorts active graphs by
glen, snake-deals them to the 8 cores (NACT slots each), and fills
inactive graphs' outputs host-side.  Per-slot arithmetic is
bit-identical to the validated v1 split-pair kernel.

Numerics: the top-3 selection margin on this generator is ~3e-5 relative
and a single selection flip costs ~2.6e-2 output rel err, so the whole
selection path (P1 token-sum, dense, q/k projections, attention scores)
keeps the proven bf16 split-pair 3-term arithmetic bit-identical to the
validated baseline.  Speed comes from data layout + scheduling:

  - All weights are host-pretiled into the exact SBUF layout so every
    weight DMA moves 3KB-contiguous per-partition segments (the naive
    rearranged loads produced 256B descriptors that capped the DMA
    queues at ~16GB/s each).
  - The clause/pooled parts of x (both D-major and node-major) are
    host-assembled into full-tile blobs: 3 large DMAs instead of 36
    fragmented ones.
  - bert hi/lo are interleaved into one tensor: one 393KB DMA per
    128-row chunk.
  - q depends only on host clause data; its 12 full-width dco quanta
    interleave with the DMA-bound P1 stream of graphs 0-3, and
    dense(h0)+k(h0) interleave with the P1 stream of graphs 4-7, so the
    PE never idles and the HAM clock gate stays released.
  - (A@x)^T is computed directly from a node-major copy of x, killing
    the per-graph xg/ax/axT transpose chains; output is written [D,NAA]
    and reassembled on host, killing 96 PE transposes + 96 copies.
"""

import numpy as np
import ml_dtypes

# ---------------- problem constants (hardcoded; kernel must be self-contained)
B_TOTAL = 64          # graphs
L = 128               # nodes per graph
HALF = 64
T = 32                # bert tokens per pair
BD = 768              # BERT_DIM == HIDDEN_DIM
D = 1536              # BD + HIDDEN
HEADS = 8
DK = D // HEADS       # 192
TOPK = 3
N_CORES = 8
DC = D // 128                     # 12 chunks of the D dim
BC = BD // 128                    # 6 chunks of the BERT dim
KCH = (HALF * T) // 128           # 16 selector k-chunks per graph
INV_SQRT_DK = 1.0 / float(np.sqrt(DK))

BF16 = ml_dtypes.bfloat16

_STATE = {}


def _split_pair(a):
    """v -> (bf16(v), bf16(v - bf16(v))) host-side split."""
    a = np.ascontiguousarray(np.asarray(a, dtype=np.float32))
    h = a.astype(BF16)
    l = (a - h.astype(np.float32)).astype(BF16)
    return h, l


def _pretile(W):
    """[K, M] -> [M//128, 128, K//128 * 128] with Wt[o, p, c*128+e] =
    W[c*128+p, o*128+e]: the exact SBUF layout of a per-dco weight tile."""
    K, M = W.shape
    Wr = np.asarray(W).reshape(K // 128, 128, M // 128, 128)
    return np.ascontiguousarray(Wr.transpose(2, 1, 0, 3).reshape(M // 128, 128, -1))


def _build_nc(G):
    """Build the per-core program for G graph slots per core."""
    GH = G // 2                   # slots per half
    NAA = G * HALF                # AA/clause rows per core
    NODES_PC = G * L              # node columns per core
    import concourse.bass as bass
    import concourse.bacc as bacc
    import concourse.mybir as mybir
    import concourse.tile as tile

    f32 = mybir.dt.float32
    bf16 = mybir.dt.bfloat16
    i32 = mybir.dt.int32
    AF = mybir.ActivationFunctionType
    OP = mybir.AluOpType

    nc = bacc.Bacc("TRN2", target_bir_lowering=False, debug=False)

    # ---------------- DRAM parameters (per-core shard shapes)
    bert_hl_d = nc.dram_tensor("bert_hl", [NAA * T, 2 * BD], bf16, kind="ExternalInput")
    xTh0_d = nc.dram_tensor("xTh0", [128, DC * NODES_PC], bf16, kind="ExternalInput")
    xTl0_d = nc.dram_tensor("xTl0", [128, DC * NODES_PC], bf16, kind="ExternalInput")
    xnm0_d = nc.dram_tensor("xnm0", [128, G * DC * 128], bf16, kind="ExternalInput")
    blen_d = nc.dram_tensor("batch_aa_bert_length", [NAA], i32, kind="ExternalInput")
    glen_d = nc.dram_tensor("aa_graph_length", [G], i32, kind="ExternalInput")
    dWhl_d = nc.dram_tensor("dense_W_hl", [BC, 128, 2 * BD], bf16, kind="ExternalInput")
    bias_d = nc.dram_tensor("biases32", [128, BC + 3 * DC], f32, kind="ExternalInput")
    Wqhl_d = nc.dram_tensor("Wq_hl", [DC, 128, 2 * D], bf16, kind="ExternalInput")
    Wkhl_d = nc.dram_tensor("Wk_hl", [DC, 128, 2 * D], bf16, kind="ExternalInput")
    gW_d = nc.dram_tensor("gcn_W16", [DC, 128, D], bf16, kind="ExternalInput")
    out_d = nc.dram_tensor("out", [D, NAA], f32, kind="ExternalOutput")

    # ---------------- inline constants
    # S0[c, r, n] = 1 where n == 4c + r//32  (selector for 4 nodes per k-chunk)
    s0 = np.zeros((KCH, 128, HALF), np.float32)
    for c in range(KCH):
        for r in range(128):
            s0[c, r, 4 * c + r // 32] = 1.0
    s0_d = nc.inline_tensor(
        np.ascontiguousarray(s0.transpose(1, 0, 2)).astype(BF16), name="s0")  # [128,16,64]
    ident_d = nc.inline_tensor(np.eye(128, dtype=np.float32), name="ident")
    ident16_d = nc.inline_tensor(np.eye(128, dtype=np.float32).astype(BF16), name="ident16")
    iota_t_d = nc.inline_tensor(
        (np.arange(128, dtype=np.float32) % T).reshape(128, 1), name="iota_t")
    iota_row_d = nc.inline_tensor(
        np.arange(128, dtype=np.float32).reshape(128, 1), name="iota_row")
    col128_d = nc.inline_tensor(
        np.broadcast_to(np.arange(128, dtype=np.float32), (128, 128)).copy(), name="col128")
    ones1_d = nc.inline_tensor(np.ones((1, 128), np.float32), name="ones1")

    with tile.TileContext(nc) as tc:
        import contextlib
        with contextlib.ExitStack() as ctx:
            cpool = ctx.enter_context(tc.tile_pool(name="const", bufs=1))
            # ---- P1-critical loads first (each small HWDGE DMA costs ~2us
            #      of serialized ring latency; phase-B consts come later)
            glen_row = cpool.tile([1, G], f32)
            glen_i = cpool.tile([1, G], i32)
            nc.sync.dma_start(out=glen_i[:], in_=glen_d.ap().unsqueeze(0))
            nc.vector.tensor_copy(out=glen_row[:], in_=glen_i[:])
            blen_row = cpool.tile([1, NAA], f32)
            blen_i = cpool.tile([1, NAA], i32)
            nc.sync.dma_start(out=blen_i[:], in_=blen_d.ap().unsqueeze(0))
            nc.vector.tensor_copy(out=blen_row[:], in_=blen_i[:])
            ones1 = cpool.tile([1, 128], f32)
            nc.sync.dma_start(out=ones1[:], in_=ones1_d.ap())
            bias_t = cpool.tile([128, BC + 3 * DC], f32)
            nc.scalar.dma_start(out=bias_t[:], in_=bias_d.ap())
            OQ, OK_, OG = BC, BC + DC, BC + 2 * DC
            db32_t = cpool.tile([128, BC], f32)
            nc.vector.tensor_scalar_mul(db32_t[:], bias_t[:, 0:BC], float(T))
            ident = cpool.tile([128, 128], f32)
            nc.scalar.dma_start(out=ident[:], in_=ident_d.ap())
            ident16 = cpool.tile([128, 128], bf16)
            nc.scalar.dma_start(out=ident16[:], in_=ident16_d.ap())
            iota_row = cpool.tile([128, 1], f32)
            nc.scalar.dma_start(out=iota_row[:], in_=iota_row_d.ap())
            col128 = cpool.tile([128, 128], f32)
            nc.scalar.dma_start(out=col128[:], in_=col128_d.ap())

            with tc.tile_pool(name="bcast_ps", bufs=2, space="PSUM") as bps:
                glen_b = cpool.tile([128, G], f32)       # graph length on every partition
                pb = bps.tile([128, G], f32)
                nc.tensor.matmul(pb[:], lhsT=ones1[:], rhs=glen_row[:], start=True, stop=True)
                nc.vector.tensor_copy(out=glen_b[:], in_=pb[:])
                blen_b = cpool.tile([128, NAA], f32)     # per-pair bert length, bcast
                for h0 in range(0, NAA, 512):
                    w = min(512, NAA - h0)
                    pb2 = bps.tile([128, 512], f32)
                    nc.tensor.matmul(pb2[:, 0:w], lhsT=ones1[:], rhs=blen_row[:, h0:h0 + w],
                                     start=True, stop=True)
                    nc.vector.tensor_copy(out=blen_b[:, h0:h0 + w], in_=pb2[:, 0:w])

            # ---------------- persistent activation tiles (bf16 pairs)
            xT_pool = ctx.enter_context(tc.tile_pool(name="xT", bufs=1))
            xTh = xT_pool.tile([128, DC, NODES_PC], bf16)   # 3.1MB
            xTl = xT_pool.tile([128, DC, NODES_PC], bf16)   # 3.1MB
            xTh4 = xTh[:].rearrange("p c (g l) -> p c g l", l=L)
            xTl4 = xTl[:].rearrange("p c (g l) -> p c g l", l=L)
            xnm = xT_pool.tile([128, GH, DC, 128], bf16)    # node-major x, per-half
            qT_pool = ctx.enter_context(tc.tile_pool(name="qT", bufs=1))
            qTh = qT_pool.tile([128, DC, G * HALF], bf16)
            qTl = qT_pool.tile([128, DC, G * HALF], bf16)
            kT_pool = ctx.enter_context(tc.tile_pool(name="kT", bufs=1))
            kTh_hf = kT_pool.tile([128, DC, GH * L], bf16)  # per-half, reused
            kTl_hf = kT_pool.tile([128, DC, GH * L], bf16)
            tokT_pool = ctx.enter_context(tc.tile_pool(name="tokT", bufs=1))
            tokTh = tokT_pool.tile([128, BC, NAA], bf16)
            tokTl = tokT_pool.tile([128, BC, NAA], bf16)
            axT_pool = ctx.enter_context(tc.tile_pool(name="axT", bufs=1))
            axT_hf = axT_pool.tile([128, DC, GH * HALF], bf16)  # per-half, reused
            denr_row = cpool.tile([1, NAA], f32)               # 1/denom per clause row

            # ---- gcn weights: persistent, preloaded during phase A (gpsimd ring)
            gw_pool = ctx.enter_context(tc.tile_pool(name="gwall", bufs=1))
            gwt_all = gw_pool.tile([128, DC, DC, 128], bf16)
            for dco in range(DC):
                nc.gpsimd.dma_start(
                    out=gwt_all[:, dco, :, :].rearrange("p c e -> p (c e)"),
                    in_=gW_d.ap()[dco])

            # ---- host-assembled blob fills (clause/pooled parts; AA-dense
            #      cols are zero and overwritten by dense later).  Per-dci
            #      pieces so q's accumulation can start as they land.
            for dci in range(DC):
                cs = slice(dci * NODES_PC, (dci + 1) * NODES_PC)
                nc.scalar.dma_start(out=xTh[:, dci, :], in_=xTh0_d.ap()[:, cs])
                nc.scalar.dma_start(out=xTl[:, dci, :], in_=xTl0_d.ap()[:, cs])

            def pair_write(dst_h, dst_l, psrc, bias=None):
                """dst_h = bf16(psrc + bias); dst_l = bf16((psrc + bias) - dst_h)."""
                if bias is None:
                    nc.scalar.copy(out=dst_h, in_=psrc)
                    nc.vector.tensor_tensor(out=dst_l, in0=psrc, in1=dst_h, op=OP.subtract)
                else:
                    nc.scalar.activation(out=dst_h, in_=psrc, func=AF.Identity,
                                         bias=bias, scale=1.0)
                    nc.vector.scalar_tensor_tensor(out=dst_l, in0=psrc, scalar=bias,
                                                   in1=dst_h, op0=OP.add, op1=OP.subtract)

            # =========================================================
            # Phase A: q (full-width) + dense/k(h0) interleaved with the
            # DMA-bound P1 BERT stream of all 8 graphs.
            # =========================================================
            p1sb = ctx.enter_context(tc.tile_pool(name="p1sb", bufs=3))
            w_pool = ctx.enter_context(tc.tile_pool(name="wblk", bufs=2))
            PS = {}

            def ps_tile(*a, **kw):
                return PS["pool"].tile(*a, **kw)
            s0_t = p1sb.tile([128, KCH, HALF], bf16, tag="s0t", bufs=1)
            nc.scalar.dma_start(out=s0_t[:], in_=s0_d.ap())
            iota_t = p1sb.tile([128, 1], f32, tag="iota_t", bufs=1)
            nc.scalar.dma_start(out=iota_t[:], in_=iota_t_d.ap())

            def load_w(Wt_, dco, dcin, nm):
                """One DMA loads the h|l pair for one dco; returns (wh, wl) views."""
                w = w_pool.tile([128, 2 * DC * 128], bf16, tag="w", name=nm, bufs=3)
                nc.scalar.dma_start(out=w[:, 0:2 * dcin * 128], in_=Wt_.ap()[dco])
                v = w[:, 0:2 * dcin * 128].rearrange("p (t c e) -> p t c e", t=2, e=128)
                return v

            p1_state = {}
            p1_blocks = {}
            CPB = 4

            def p1_chunk(g, c):
                ci_g = g * KCH + c
                bi = ci_g // CPB
                if ci_g % CPB == 0:
                    blk = p1sb.tile([128, CPB, 2 * BD], bf16, tag="bblk",
                                    name="bblk", bufs=3)
                    r0 = bi * (CPB * 128)
                    nc.sync.dma_start(
                        out=blk[:],
                        in_=bert_hl_d.ap()[r0:r0 + CPB * 128, :].rearrange(
                            "(c p) d -> p c d", p=128))
                    p1_blocks[bi] = blk
                if c == 0:
                    lt = p1sb.tile([128, HALF], bf16, tag="lt", name="lt", bufs=2)
                    nc.vector.tensor_tensor(
                        out=lt[:], in0=blen_b[:, g * HALF:(g + 1) * HALF],
                        in1=iota_t[:].to_broadcast([128, HALF]), op=OP.is_gt)
                    ptok = ps_tile([64, BD], f32, tag="ptok", name="ptok", bufs=2)
                    p1_state[g] = (lt, ptok)
                lt, ptok = p1_state[g]
                blk = p1_blocks[(g * KCH + c) // CPB]
                cc = (g * KCH + c) % CPB
                sm = p1sb.tile([128, HALF], bf16, tag="sm", name="sm", bufs=3)
                nc.vector.tensor_tensor(out=sm[:], in0=s0_t[:, c, :], in1=lt[:],
                                        op=OP.mult)
                first, last = (c == 0), (c == KCH - 1)
                nc.tensor.matmul(ptok[:, 0:512], lhsT=sm[:], rhs=blk[:, cc, 0:512],
                                 start=first, stop=False)
                nc.tensor.matmul(ptok[:, 0:512], lhsT=sm[:], rhs=blk[:, cc, BD:BD + 512],
                                 start=False, stop=last)
                nc.tensor.matmul(ptok[:, 512:768], lhsT=sm[:], rhs=blk[:, cc, 512:768],
                                 start=first, stop=False)
                nc.tensor.matmul(ptok[:, 512:768], lhsT=sm[:],
                                 rhs=blk[:, cc, BD + 512:2 * BD],
                                 start=False, stop=last)
                if last:
                    tok_ng = p1sb.tile([64, BD], f32, tag="tokng", name="tokng", bufs=2)
                    nc.scalar.copy(out=tok_ng[:], in_=ptok[:])
                    ptr6 = ps_tile([128, BC * HALF], f32, tag="ptr6", name="ptr6",
                                        bufs=2)
                    for dcH in range(BC):
                        nc.tensor.matmul(ptr6[:, dcH * HALF:(dcH + 1) * HALF],
                                         lhsT=tok_ng[:, dcH * 128:(dcH + 1) * 128],
                                         rhs=ident[0:64, 0:64], start=True, stop=True,
                                         is_transpose=True)
                    pair_write(tokTh[:, :, g * HALF:(g + 1) * HALF],
                               tokTl[:, :, g * HALF:(g + 1) * HALF],
                               ptr6[:].rearrange("p (c n) -> p c n", n=HALF))
                    del p1_state[g]

            def q_quantum(dco):
                wv = load_w(Wqhl_d, dco, DC, "wq")
                pq = ps_tile([128, G * HALF], f32, tag="pq", name="pq", bufs=2)
                for dci in range(DC):
                    nc.tensor.matmul(pq[:], lhsT=wv[:, 0, dci, :],
                                     rhs=xTh4[:, dci, :, HALF:L],
                                     start=(dci == 0), stop=False)
                    nc.tensor.matmul(pq[:], lhsT=wv[:, 0, dci, :],
                                     rhs=xTl4[:, dci, :, HALF:L],
                                     start=False, stop=False)
                    nc.tensor.matmul(pq[:], lhsT=wv[:, 1, dci, :],
                                     rhs=xTh4[:, dci, :, HALF:L],
                                     start=False, stop=(dci == DC - 1))
                pair_write(qTh[:, dco, :], qTl[:, dco, :], pq[:],
                           bias=bias_t[:, OQ + dco:OQ + dco + 1])

            def dense_quantum(hf, dco):
                gs = slice(hf * GH, (hf + 1) * GH)
                ts_ = slice(hf * GH * HALF, (hf + 1) * GH * HALF)
                dwv = load_w(dWhl_d, dco, BC, "wd")
                pdt = ps_tile([128, 512], f32, tag="pq", name="pd", bufs=2)
                pd = pdt[:, 0:GH * HALF]
                for dci in range(BC):
                    nc.tensor.matmul(pd, lhsT=dwv[:, 0, dci, :],
                                     rhs=tokTh[:, dci, ts_],
                                     start=(dci == 0), stop=False)
                    nc.tensor.matmul(pd, lhsT=dwv[:, 0, dci, :],
                                     rhs=tokTl[:, dci, ts_], start=False, stop=False)
                    nc.tensor.matmul(pd, lhsT=dwv[:, 1, dci, :],
                                     rhs=tokTh[:, dci, ts_], start=False,
                                     stop=(dci == BC - 1))
                pair_write(xTh4[:, dco, gs, 0:HALF], xTl4[:, dco, gs, 0:HALF],
                           pd.rearrange("p (g h) -> p g h", h=HALF),
                           bias=db32_t[:, dco:dco + 1])

            def xnm_fill(hf):
                # host clause/pooled parts for this half, then node-major
                # transposes of the dense part of x
                hs = slice(hf * GH * DC * 128, (hf + 1) * GH * DC * 128)
                nc.scalar.dma_start(out=xnm[:].rearrange("p g c e -> p (g c e)"),
                                    in_=xnm0_d.ap()[:, hs])
                for g in range(hf * GH, (hf + 1) * GH):
                    gl = g % GH
                    pnm = ps_tile([64, BC * 128], bf16, tag="pnm", name="pnm",
                                       bufs=1)
                    for dco in range(BC):
                        nc.tensor.matmul(pnm[:, dco * 128:(dco + 1) * 128],
                                         lhsT=xTh4[:, dco, g, 0:HALF],
                                         rhs=ident16[:], start=True, stop=True,
                                         is_transpose=True)
                    nc.scalar.copy(out=xnm[0:HALF, gl, 0:BC, :],
                                   in_=pnm[:].rearrange("p (c e) -> p c e", e=128))

            def k_quantum(hf, dco):
                nsl = slice(hf * GH * L, (hf + 1) * GH * L)
                wv = load_w(Wkhl_d, dco, DC, "wk")
                pk = ps_tile([128, GH * L], f32, tag="pq", name="pk", bufs=2)
                for dci in range(DC):
                    nc.tensor.matmul(pk[:], lhsT=wv[:, 0, dci, :],
                                     rhs=xTh[:, dci, nsl],
                                     start=(dci == 0), stop=False)
                    nc.tensor.matmul(pk[:], lhsT=wv[:, 0, dci, :],
                                     rhs=xTl[:, dci, nsl],
                                     start=False, stop=False)
                    nc.tensor.matmul(pk[:], lhsT=wv[:, 1, dci, :],
                                     rhs=xTh[:, dci, nsl],
                                     start=False, stop=(dci == DC - 1))
                pair_write(kTh_hf[:, dco, :], kTl_hf[:, dco, :], pk[:],
                           bias=bias_t[:, OK_ + dco:OK_ + dco + 1])

            # ---- interleave schedule for phase A
            chunks = [(g, c) for g in range(G) for c in range(KCH)]
            ci = 0
            with tc.tile_pool(name="psA", bufs=1, space="PSUM") as psA:
                PS["pool"] = psA
                # pre-roll: keep PE busy while the xT pieces land
                while ci < 12:
                    p1_chunk(*chunks[ci])
                    ci += 1
                # q dcos paced against P1 of graphs 0..3 (64 chunks)
                for qi in range(DC):
                    q_quantum(qi)
                    target = 12 + (qi + 1) * (GH * KCH - 12) // DC
                    while ci < target:
                        p1_chunk(*chunks[ci])
                        ci += 1
                # dense(h0) + k(h0) paced against P1 of graphs 4..7
                abq = [("d", 0, dco) for dco in range(BC)] + \
                      [("k", 0, dco) for dco in range(DC)]
                for bi, (kind, hf, dco) in enumerate(abq):
                    if kind == "d":
                        dense_quantum(hf, dco)
                    else:
                        k_quantum(hf, dco)
                    target = GH * KCH + (bi + 1) * (GH * KCH) // len(abq)
                    while ci < target:
                        p1_chunk(*chunks[ci])
                        ci += 1

            # =========================================================
            # Phase B: dense(h1), scores(h0), k(h1), scores(h1),
            # axT+GCN per half overlapping the DVE softmax chains.
            # =========================================================
            p3sb = ctx.enter_context(tc.tile_pool(name="p3sb", bufs=2))
            gcn_pool = ctx.enter_context(tc.tile_pool(name="gcnw", bufs=2))
            ost_pool = ctx.enter_context(tc.tile_pool(name="ostg", bufs=2))

            def scores_graph(g):
                """Masked 8-head attention + head-mean + top-3 for graph g."""
                gl = g % GH
                colmask = p3sb.tile([128, L], f32, tag="colmask", bufs=2)
                nc.vector.tensor_scalar(
                    out=colmask[:], in0=col128[:], scalar1=glen_b[:, g:g + 1],
                    scalar2=-1e9, op0=OP.is_ge, op1=OP.mult)
                vrow = p3sb.tile([128, 1], f32, tag="vrow", bufs=2)
                nc.vector.tensor_scalar(
                    out=vrow[:], in0=iota_row[:], scalar1=glen_b[:, g:g + 1],
                    scalar2=1.0 / HEADS, op0=OP.is_lt, op1=OP.mult)
                adj = p3sb.tile([64, L], f32, tag="adj", bufs=2)
                ps4 = None
                for h in range(HEADS):
                    if h % 4 == 0:
                        ps4 = ps_tile([64, 4 * L], f32, tag="pscore", name="ps4", bufs=2)
                    ps = ps4[:, (h % 4) * L:(h % 4 + 1) * L]
                    r0 = h * DK
                    hchunks = [(ra, rb) for (ra, rb) in
                               ((r0, min(r0 + DK, (r0 // 128 + 1) * 128)),
                                ((r0 // 128 + 1) * 128, r0 + DK)) if rb > ra]
                    for hci, (ra, rb) in enumerate(hchunks):
                        tI, p0 = ra // 128, ra % 128
                        p1_ = p0 + (rb - ra)
                        qs = slice(g * HALF, (g + 1) * HALF)
                        ks = slice(gl * L, (gl + 1) * L)
                        first = hci == 0
                        last = hci == len(hchunks) - 1
                        nc.tensor.matmul(ps, lhsT=qTh[p0:p1_, tI, qs],
                                         rhs=kTh_hf[p0:p1_, tI, ks],
                                         start=first, stop=False)
                        nc.tensor.matmul(ps, lhsT=qTh[p0:p1_, tI, qs],
                                         rhs=kTl_hf[p0:p1_, tI, ks],
                                         start=False, stop=False)
                        nc.tensor.matmul(ps, lhsT=qTl[p0:p1_, tI, qs],
                                         rhs=kTh_hf[p0:p1_, tI, ks],
                                         start=False, stop=last)
                    # mask invalid key columns; softmax over keys of s/sqrt(dk)
                    nc.vector.tensor_tensor(out=ps, in0=ps, in1=colmask[0:64, :],
                                            op=OP.add)
                    negmax = p3sb.tile([64, 1], f32, tag="negmax", bufs=3)
                    nc.vector.reduce_max(out=negmax[:], in_=ps,
                                         axis=mybir.AxisListType.X, negate=True)
                    nms = p3sb.tile([64, 1], f32, tag="nms", bufs=3)
                    nc.vector.tensor_scalar_mul(nms[:], negmax[:], INV_SQRT_DK)
                    exph = p3sb.tile([64, L], f32, tag="exph", bufs=3)
                    sumexp = p3sb.tile([64, 1], f32, tag="sumexp", bufs=3)
                    nc.scalar.activation(out=exph[:], in_=ps, func=AF.Exp,
                                         bias=nms[:], scale=INV_SQRT_DK,
                                         accum_out=sumexp[:])
                    recip = p3sb.tile([64, 1], f32, tag="recip", bufs=3)
                    nc.vector.reciprocal(out=recip[:], in_=sumexp[:])
                    if h == 0:
                        nc.vector.tensor_scalar(out=adj[:], in0=exph[:], scalar1=recip[:],
                                                scalar2=None, op0=OP.mult)
                    else:
                        nc.vector.scalar_tensor_tensor(
                            out=adj[:], in0=exph[:], scalar=recip[:], in1=adj[:],
                            op0=OP.mult, op1=OP.add)
                nc.vector.tensor_scalar(out=adj[:], in0=adj[:], scalar1=vrow[64:128, :],
                                        scalar2=None, op0=OP.mult)
                # top-3 selection
                top8 = p3sb.tile([64, 8], f32, tag="top8", bufs=2)
                nc.vector.max(out=top8[:], in_=adj[:])
                nc.vector.memset(top8[:, TOPK:8], 0.0)
                zapped = p3sb.tile([64, L], f32, tag="zapped", bufs=2)
                nc.vector.match_replace(out=zapped[:], in_to_replace=top8[:],
                                        in_values=adj[:], imm_value=0.0)
                adjsel = p3sb.tile([64, L], f32, tag="adjsel", bufs=5)
                denom = p3sb.tile([64, 1], f32, tag="denom", bufs=5)
                nc.vector.tensor_tensor(out=adjsel[:], in0=adj[:], in1=zapped[:],
                                        op=OP.subtract)
                nc.vector.reduce_sum(out=denom[:], in_=adjsel[:],
                                     axis=mybir.AxisListType.X)
                nc.vector.tensor_scalar_add(denom[:], denom[:], 1.0)
                recip_d = p3sb.tile([64, 1], f32, tag="recipd", bufs=5)
                nc.vector.reciprocal(out=recip_d[:], in_=denom[:])
                return adjsel, recip_d

            def axT_graph(g, adjsel, recip_d):
                """adjT transpose + denom row + (A@x)^T directly from xnm."""
                gl = g % GH
                mix0 = ps_tile([128, 512], f32, tag="p3mix", name="mix0", bufs=2)
                pat = mix0[:, 0:64]
                nc.tensor.matmul(pat, lhsT=adjsel[:], rhs=ident[0:64, 0:64],
                                 start=True, stop=True, is_transpose=True)
                adjT16 = p3sb.tile([128, 64], bf16, tag="adjT16", bufs=2)
                nc.vector.tensor_copy(out=adjT16[:], in_=pat)
                prd = mix0[0:1, 64:128]
                nc.tensor.matmul(prd, lhsT=recip_d[:], rhs=ident[0:64, 0:64],
                                 start=True, stop=True, is_transpose=True)
                nc.vector.tensor_copy(out=denr_row[:, g * HALF:(g + 1) * HALF], in_=prd)
                for half6 in range(2):
                    mix = mix0 if half6 == 0 else ps_tile([128, 512], f32,
                                                               tag="p3mix",
                                                               name="mix1", bufs=2)
                    paxT = mix[:, 128:128 + 6 * HALF]
                    for j in range(6):
                        dc = half6 * 6 + j
                        nc.tensor.matmul(paxT[:, j * HALF:(j + 1) * HALF],
                                         lhsT=xnm[:, gl, dc, :], rhs=adjT16[:],
                                         start=True, stop=True)
                    nc.scalar.copy(
                        out=axT_hf[:, half6 * 6:(half6 + 1) * 6,
                                   gl * HALF:(gl + 1) * HALF],
                        in_=paxT.rearrange("p (c n) -> p c n", n=HALF))

            def gcn_prep(hf):
                """Broadcast 1/denom across partitions for one half."""
                denrb = p3sb.tile([128, GH * HALF], f32, tag="denrb", bufs=2)
                pb3t = ps_tile([128, 512], f32, tag="pq", name="pdenb", bufs=2)
                pb3 = pb3t[:, 0:GH * HALF]
                nc.tensor.matmul(pb3, lhsT=ones1[:],
                                 rhs=denr_row[:, hf * GH * HALF:(hf + 1) * GH * HALF],
                                 start=True, stop=True)
                nc.vector.tensor_copy(out=denrb[:], in_=pb3)
                return denrb

            def gcn_dco(hf, dco, denrb):
                """One GCN output-dim quantum + relu/denom + DMA out."""
                pgt = ps_tile([128, 512], f32, tag="pq", name="pg", bufs=2)
                pg = pgt[:, 0:GH * HALF]
                for dci in range(DC):
                    nc.tensor.matmul(pg, lhsT=gwt_all[:, dco, dci, :],
                                     rhs=axT_hf[:, dci, :],
                                     start=(dci == 0), stop=(dci == DC - 1))
                trel = ost_pool.tile([128, GH * HALF], f32, tag="trel")
                nc.scalar.activation(out=trel[:], in_=pg, func=AF.Relu,
                                     bias=bias_t[:, OG + dco:OG + dco + 1], scale=1.0)
                ostg = ost_pool.tile([128, GH * HALF], f32, tag="ostg")
                nc.vector.tensor_tensor(out=ostg[:], in0=trel[:], in1=denrb[:],
                                        op=OP.mult)
                nc.sync.dma_start(
                    out=out_d.ap()[dco * 128:(dco + 1) * 128,
                                   hf * GH * HALF:(hf + 1) * GH * HALF],
                    in_=ostg[:])

            # ---- phase B schedule
            with tc.tile_pool(name="psB", bufs=1, space="PSUM") as psB:
                PS["pool"] = psB
                sel0 = []
                xnm_fill(0)
                for dco in range(BC):
                    dense_quantum(1, dco)
                # scores(h0) first so k(h1) PE work overlaps their DVE chains
                for g in range(GH):
                    sel0.append(scores_graph(g))
                for dco in range(0, DC // 2):
                    k_quantum(1, dco)
                for g in range(GH):
                    axT_graph(g, *sel0[g])
                for dco in range(DC // 2, DC):
                    k_quantum(1, dco)
                xnm_fill(1)
                den0 = gcn_prep(0)
                sel1 = []
                sel1.append(scores_graph(GH))
                for dco in range(0, DC // 2):
                    gcn_dco(0, dco, den0)
                sel1.append(scores_graph(GH + 1))
                for dco in range(DC // 2, DC):
                    gcn_dco(0, dco, den0)
                for g in range(GH, G):
                    axT_graph(g, *sel1[g - GH])
                den1 = gcn_prep(1)
                for dco in range(DC):
                    gcn_dco(1, dco, den1)

    nc.compile()
    return nc


def _get_nc(G):
    key = ("nc", G)
    if key not in _STATE:
        _STATE[key] = _build_nc(G)
    return _STATE[key]


def _plan(glen_full):
    """Sort active graphs (glen > 64) by glen desc, snake-deal to cores.

    Returns (G_slots, slots) where slots[c][j] is a global graph id or None."""
    order = np.argsort(-glen_full, kind="stable")
    active = [int(g) for g in order if glen_full[g] > 64]
    n_act = len(active)
    if n_act == 0:
        return 0, None
    import math
    G = max(2, 2 * math.ceil(math.ceil(n_act / N_CORES) / 2))
    slots = [[None] * G for _ in range(N_CORES)]
    i = 0
    for r in range(G):
        cs = range(N_CORES) if r % 2 == 0 else reversed(range(N_CORES))
        for c in cs:
            if i < n_act:
                slots[c][r] = active[i]
                i += 1
    return G, slots


def _shard_inputs(inputs, G, slots):
    """Per-core input maps for the G-slot program: gather the slot graphs'
    rows (zeros for dummy slots), split bf16 pairs, pretile weights."""
    GH = G // 2
    NAA = G * HALF
    bert = np.asarray(inputs["inner_bert_out"], dtype=np.float32).reshape(
        B_TOTAL, HALF * T, BD)
    pooled = np.asarray(inputs["inner_pooled_out"], dtype=np.float32).reshape(
        B_TOTAL, HALF, BD)
    clause = np.asarray(inputs["clause_output"], dtype=np.float32).reshape(
        B_TOTAL, HALF, D)
    blen = np.asarray(inputs["batch_aa_bert_length"], dtype=np.int32).reshape(
        B_TOTAL, HALF)
    glen = np.ascontiguousarray(np.asarray(inputs["aa_graph_length"], dtype=np.int32))
    dWh, dWl = _split_pair(inputs["dense_W"])
    Wqh, Wql = _split_pair(inputs["Wq"])
    Wkh, Wkl = _split_pair(inputs["Wk"])
    gW16 = np.asarray(inputs["gcn_W"], dtype=np.float32).astype(BF16)
    reps = {
        "dense_W_hl": np.ascontiguousarray(
            np.concatenate([_pretile(dWh), _pretile(dWl)], axis=2)),

        "Wq_hl": np.ascontiguousarray(
            np.concatenate([_pretile(Wqh), _pretile(Wql)], axis=2)),

        "Wk_hl": np.ascontiguousarray(
            np.concatenate([_pretile(Wkh), _pretile(Wkl)], axis=2)),

        "gcn_W16": _pretile(gW16),
        "biases32": np.ascontiguousarray(np.concatenate([
            np.asarray(inputs["dense_b"], np.float32).reshape(BC, 128).T,
            np.asarray(inputs["bq"], np.float32).reshape(DC, 128).T,
            np.asarray(inputs["bk"], np.float32).reshape(DC, 128).T,
            np.asarray(inputs["gcn_b"], np.float32).reshape(DC, 128).T,
        ], axis=1)),
    }
    in_maps = []
    for c in range(N_CORES):
        gs = slots[c]
        csel = np.zeros((G, HALF, D), np.float32)
        psel = np.zeros((G, HALF, BD), np.float32)
        bsel = np.zeros((G, HALF * T, BD), np.float32)
        blsel = np.zeros((G, HALF), np.int32)
        glsel = np.zeros((G,), np.int32)
        for j, g in enumerate(gs):
            if g is None:
                continue
            csel[j] = clause[g]
            psel[j] = pooled[g]
            bsel[j] = bert[g]
            blsel[j] = blen[g]
            glsel[j] = glen[g]
        ch, cl = _split_pair(csel.reshape(NAA, D))
        ph, pl = _split_pair(psel.reshape(NAA, BD))
        bh, bl = _split_pair(bsel.reshape(NAA * T, BD))
        bert_hl = np.concatenate([bh, bl], axis=1)        # [NAA*T, 2*BD]

        # xT blobs [128, DC, G, L]: clause at [:, :, :, 64:], pooled at [6:, :, :64]
        def xt_blob(chh, phh):
            b = np.zeros((128, DC, G, L), BF16)
            b[:, :, :, HALF:] = chh.reshape(G, HALF, DC, 128).transpose(3, 2, 0, 1)
            b[:, BC:, :, 0:HALF] = phh.reshape(G, HALF, BC, 128).transpose(3, 2, 0, 1)
            return np.ascontiguousarray(b.reshape(128, -1))

        # xnm blob [128 nodes, G, DC, 128]: clause rows at [64:], pooled at [0:64, :, 6:]
        xnmb = np.zeros((128, G, DC, 128), BF16)
        xnmb[HALF:] = ch.reshape(G, HALF, DC, 128).transpose(1, 0, 2, 3)
        xnmb[0:HALF, :, BC:] = ph.reshape(G, HALF, BC, 128).transpose(1, 0, 2, 3)
        m = {
            "bert_hl": bert_hl,
            "xTh0": xt_blob(ch, ph), "xTl0": xt_blob(cl, pl),
            "xnm0": np.ascontiguousarray(xnmb.reshape(128, -1)),
            "batch_aa_bert_length": blsel.reshape(NAA),
            "aa_graph_length": glsel,
        }
        m.update(reps)
        in_maps.append(m)
    return in_maps


def _assemble(inputs, G, slots, results):
    """Scatter per-core slot outputs back to the full [B*HALF, D] tensor;
    fill inactive graphs host-side."""
    glen = np.asarray(inputs["aa_graph_length"], dtype=np.int64)
    clause = np.asarray(inputs["clause_output"], dtype=np.float32)
    gb = np.asarray(inputs["gcn_b"], dtype=np.float32)
    out = np.zeros((B_TOTAL * HALF, D), np.float32)
    base = np.maximum(gb, 0.0)[None, :]               # gcn row for zero adj
    for g in range(B_TOTAL):
        r0 = g * HALF
        if glen[g] <= 1:
            out[r0:r0 + HALF] = clause[r0:r0 + HALF]
        elif glen[g] <= HALF:
            out[r0:r0 + HALF] = base
    if slots is not None:
        for c in range(N_CORES):
            oc = np.asarray(results[c]["out"], dtype=np.float32)   # [D, NAA]
            for j, g in enumerate(slots[c]):
                if g is None:
                    continue
                out[g * HALF:(g + 1) * HALF] = oc[:, j * HALF:(j + 1) * HALF].T
    return out


def run_sharded(inputs, trace=False):
    from concourse.bass_utils import run_bass_kernel_spmd

    glen_full = np.asarray(inputs["aa_graph_length"], dtype=np.int64)
    G, slots = _plan(glen_full)
    if G == 0:
        return _assemble(inputs, G, None, None), None
    nc = _get_nc(G)
    in_maps = _shard_inputs(inputs, G, slots)
    res = run_bass_kernel_spmd(nc, in_maps, core_ids=list(range(N_CORES)),
                               trace=trace)
    return _assemble(inputs, G, slots, res.results), res


def kernel(**inputs) -> np.ndarray:
    out, _ = run_sharded(inputs)
    return out
